# revision 1
# baseline (speedup 1.0000x reference)
"""ECE (expected calibration error) kernel for Trainium2, 8 NeuronCores.

Math (matches torch ECELoss(n_bins=20) / the jax reference):
    conf_i = max_c outputs[i, c]
    acc_i  = 1[outputs[i, labels_i] == conf_i]   (== argmax correct; exact on
             this data - verified zero tie mismatches)
    bin membership via step functions S[i, b] = conf_i > b/20, b = 0..20
    cum[b] = sum_i S[i,b] * v_i  for v in {conf, acc}
    sum_v[b] = cum[b] - cum[b+1]         (equal-width (lo, hi] bins + clip)
    ece = sum_b |sum_conf[b] - sum_acc[b]| / N

Device mapping (per core, data-parallel over samples):
    - input arranged [P=128 partitions, JR rows, C=128 classes]; tile = 128
      samples x 128 classes; groups of G tiles per DMA (contiguous per
      partition).
    - VectorE: batched reduce_max over a group -> conf; per tile one
      scalar_tensor_tensor (iota == label) * x with accum_out -> picked =
      x[i, label] in a single pass (STT only exists on VectorE here).
    - GPSIMD: acc = (picked == conf) and S[i,b] = (conf > edge_b), each as
      TT-subtract + TS-compare-vs-0 (Pool TT comparisons don't lower on
      this toolchain; fp32 subtraction is sign-exact so this is identical).
    - TensorE: per-jumbo matmul [K=128] x ([2J] x [J*(NB+1)]) accumulating
      cum partial sums into PSUM across the whole shard.
    - host: sum the 8 cores' [2J, J*(NB+1)] partials, undo the jumbo
      cross-product layout, finish the 21->20 differencing and |.|/N.
    Measured ~360 us per core-shard pass (65.5 MB/core read) vs the ~183 us
    per-core HBM roofline, with VectorE (conf pass + 20 STT gathers) the
    bottleneck engine.
Padding rows are all-zero => conf = 0 => S == 0 => they contribute nothing.

Built on bacc.Bacc (not raw Bass): its compile pipeline legalizes
multi-sync-wait instructions via event semaphores, which this walrus build
requires (each ISA struct carries only one sync wait).
"""

import numpy as np

P = 128          # SBUF partitions (samples per tile)
C = 128          # classes
NB = 20          # ECE bins
NE = NB + 1      # bin edges
NCORES = 8
G = 20           # tiles per group (per DMA / per batched vector op)
                 # (G=40 measured: correct but slower — bigger x tiles hurt
                 # SBUF overlap more than the halved fixed costs help)
J = 10           # tiles per jumbo matmul (M = 2*J <= 128, N = J*NE <= 512)
N_DVE = 6        # how many of the G picked-gathers run on VectorE (rest GPSIMD)


def _get_winop():
    """Register (once) a custom DVE op: out = (C0 <= Idx < C1) * Src0,
    accum_out = sum(out). Single tensor input -> eligible for the fp32
    2x perf mode, unlike the two-input scalar_tensor_tensor gather."""
    import concourse.dve_ops as dvo

    for op in dvo.OPS:
        if op.name == "TENSOR_WINDOW_SUM_ANT":
            return op
    from operator import add

    import numpy as np_
    from concourse.dve_spec import C0, C1, Idx, Spec, Src0, Zero

    def ref(in0, in1, c0, c1, c2):
        p = in0.shape[0]
        x = in0.astype(np_.float32).reshape(p, -1)
        idx = np_.broadcast_to(
            np_.arange(x.shape[1], dtype=np_.float32), x.shape
        )
        b = (((idx >= c0) & (idx < c1)).astype(np_.float32) * x).astype(
            np_.float32
        )
        return b, b.sum(axis=-1, keepdims=True)

    op = dvo.DveOp(
        "TENSOR_WINDOW_SUM_ANT",
        Spec(
            body=((Idx >= C0) & (Idx < C1)) * Src0,
            accum=add,
            accum_init=Zero,
            reference=ref,
        ),
        subdim=False,
        uops_sha={"v3": "643c66c31669334b"},
        perf_en={"v3": True},
    )
    dvo.OPS.append(op)
    dvo._SUB_OPCODE_FOR_NAME[op.name] = (
        max(dvo._SUB_OPCODE_FOR_NAME.values()) + 1
    )
    dvo.CUSTOM_DVE_SPECS[op.name] = op.spec
    return op


def build_nc(jr, n_dve=N_DVE, repeat=1, do_stt=True, do_small=True,
             gather="stt"):
    """Build the Bass module for one core with JR rows per partition.

    repeat > 1 wraps the whole group loop in an on-device For_i that
    recomputes the same result `repeat` times (PSUM restarts each trip) —
    used only for wall-clock perf measurement via run-time deltas.
    """
    import contextlib

    import concourse.bacc as bacc
    import concourse.mybir as mybir
    from concourse.tile import TileContext

    f32 = mybir.dt.float32
    Alu = mybir.AluOpType
    ng = jr // G
    assert jr % G == 0 and G % J == 0
    nj = G // J

    nc = bacc.Bacc("TRN2", target_bir_lowering=False)
    x = nc.dram_tensor("x", (P, jr, C), f32, kind="ExternalInput")
    # one consts tensor = one DMA = one completion semaphore
    consts = nc.dram_tensor(
        "consts", (P, NE + C + jr), f32, kind="ExternalInput"
    )
    out = nc.dram_tensor("out", (2 * J, NE * J), f32, kind="ExternalOutput")

    with TileContext(nc) as tc:
        with (
            tc.tile_pool(name="consts", bufs=1) as cpool,
            tc.tile_pool(name="xin", bufs=4) as xpool,
            tc.tile_pool(name="vt", bufs=3) as vpool,
            tc.tile_pool(name="pk", bufs=3) as kpool,
            tc.tile_pool(name="st", bufs=3) as spool,
            tc.tile_pool(name="scrv", bufs=4) as scrvpool,
            tc.tile_pool(name="scrg", bufs=2) as scrgpool,
            tc.tile_pool(name="res", bufs=1) as rpool,
            tc.tile_pool(name="acc", bufs=1, space="PSUM") as ppool,
        ):
            constsb = cpool.tile([P, NE + C + jr], f32)
            nc.sync.dma_start(constsb[:], consts[:])
            edgesb = constsb[:][:, 0:NE]
            iotasb = constsb[:][:, NE:NE + C]
            labsb = constsb[:][:, NE + C:]
            if gather in ("tmr", "win"):
                # labels + 1 (window end)
                labp1 = cpool.tile([P, jr], f32)
                nc.vector.tensor_scalar_add(labp1[:], labsb, 1.0)
            winop = _get_winop() if gather == "win" else None

            psum = ppool.tile([2 * J, NE * J], f32)

            def group_body(g):
                xt = xpool.tile([P, G, C], f32)
                nc.sync.dma_start(xt[:], x[:, g * G:(g + 1) * G, :])

                # vt free layout: per jumbo j a contiguous [conf(J) | acc(J)]
                # block, so each matmul's stationary AP is one free dim.
                vt = vpool.tile([P, nj, 2 * J], f32)
                vt4 = vt[:].rearrange("p j (h t) -> p j h t", h=2)
                if not do_small:
                    nc.vector.memset(vt[:], 0.0)
                nc.vector.tensor_reduce(
                    vt4[:, :, 0, :], xt[:], axis=mybir.AxisListType.X, op=Alu.max
                )

                # picked[i, t] = x[i, label] : (iota == lab)*x, accum-summed.
                # STT only exists on VectorE (Pool fails the engine check).
                pk = kpool.tile([P, G], f32)
                for t in range(G if do_stt else 0):
                    scr = scrvpool.tile([P, C], f32)
                    if gather == "win":
                        nc.vector._custom_dve(
                            winop,
                            out=scr[:],
                            in0=xt[:][:, t, :],
                            s0=labsb[:, g * G + t: g * G + t + 1],
                            s1=labp1[:][:, g * G + t: g * G + t + 1],
                            accum_out=pk[:][:, t: t + 1],
                        )
                    elif gather == "tmr":
                        # picked = max over the [label, label+1) window
                        nc.vector.tensor_mask_reduce(
                            scr[:],
                            xt[:][:, t, :],
                            labsb[:, g * G + t: g * G + t + 1],
                            labp1[:][:, g * G + t: g * G + t + 1],
                            1.0,
                            -3.0e38,
                            Alu.max,
                            accum_out=pk[:][:, t: t + 1],
                        )
                    else:
                        nc.vector.scalar_tensor_tensor(
                            scr[:],
                            iotasb,
                            labsb[:, g * G + t: g * G + t + 1],
                            xt[:][:, t, :],
                            op0=Alu.is_equal,
                            op1=Alu.mult,
                            accum_out=pk[:][:, t: t + 1],
                        )

                # Pool: acc = (picked == conf), via subtract + compare-to-0
                # (Pool TT supports arithmetic ops only; TS supports cmp).
                # fp32 subtraction is sign-exact, so this matches is_equal.
                pk3 = pk[:].rearrange("p (j t) -> p j t", j=nj)
                st = spool.tile([P, G, NE], f32)
                st4 = st[:].rearrange("p (j t) e -> p j t e", j=nj)
                if not do_stt and do_small:
                    nc.vector.memset(pk[:], 0.0)
                if not do_small:
                    nc.vector.memset(st[:], 1.0)
                if do_small:
                    nc.gpsimd.tensor_tensor(
                        vt4[:, :, 1, :], pk3, vt4[:, :, 0, :], Alu.subtract
                    )
                    nc.gpsimd.tensor_scalar(
                        vt4[:, :, 1, :], vt4[:, :, 1, :], 0.0, None, Alu.is_equal
                    )

                    # Pool: S[i, t, b] = conf[i, t] > edge[b], same trick
                    conf4 = vt4[:, :, 0, :][:, :, :, None].broadcast_to(
                        [P, nj, J, NE]
                    )
                    edges4 = edgesb[:, None, None, :].broadcast_to(
                        [P, nj, J, NE]
                    )
                    nc.gpsimd.tensor_tensor(st4, conf4, edges4, Alu.subtract)
                    nc.gpsimd.tensor_scalar(st4, st4, 0.0, None, Alu.is_gt)

                # PE: accumulate cum[(h,t), (t',b)] += sum_i V[i,h,t]*S[i,t',b]
                for j in range(nj):
                    nc.tensor.matmul(
                        psum[:],
                        vt[:][:, j, :],
                        st[:][:, j * J:(j + 1) * J, :],
                        start=(g == 0 and j == 0),
                        stop=(g == ng - 1 and j == nj - 1),
                    )

            if repeat > 1:
                with tc.For_i(0, repeat, 1):
                    for g in range(ng):
                        group_body(g)
            else:
                for g in range(ng):
                    group_body(g)

            res = rpool.tile([2 * J, NE * J], f32)
            nc.scalar.copy(res[:], psum[:])
            nc.sync.dma_start(out[:], res[:])

    nc.finalize()
    return nc


def _prep_inputs(outputs, labels, ncores, jr):
    cap = ncores * P * jr
    n = outputs.shape[0]
    xpad = np.zeros((cap, C), np.float32)
    xpad[:n] = outputs
    lpad = np.zeros((cap,), np.float32)
    lpad[:n] = labels.astype(np.float32)
    xs = xpad.reshape(ncores, P, jr, C)
    ls = lpad.reshape(ncores, P, jr)
    consts = np.empty((ncores, P, NE + C + jr), np.float32)
    consts[:, :, 0:NE] = (np.arange(NE, dtype=np.float32) / NB).astype(
        np.float32
    )
    consts[:, :, NE:NE + C] = np.arange(C, dtype=np.float32)
    consts[:, :, NE + C:] = ls
    return [{"x": xs[c], "consts": consts[c]} for c in range(ncores)]


def _decode(core_outs, n):
    acc = np.zeros((2 * J, NE * J), np.float64)
    for r in core_outs:
        acc += r
    cum_conf = np.zeros(NE, np.float64)
    cum_acc = np.zeros(NE, np.float64)
    for k in range(J):
        cum_conf += acc[k, k * NE:(k + 1) * NE]
        cum_acc += acc[J + k, k * NE:(k + 1) * NE]
    sum_conf = cum_conf[:NB] - cum_conf[1:]
    sum_acc = cum_acc[:NB] - cum_acc[1:]
    ece = np.abs(sum_conf - sum_acc).sum() / n
    return np.array([ece], dtype=np.float32)


def kernel_impl(outputs, labels, trace=False):
    from concourse import bass_utils

    outputs = np.ascontiguousarray(np.asarray(outputs), dtype=np.float32)
    labels = np.asarray(labels)
    n = outputs.shape[0]
    assert outputs.shape[1] == C
    jr = -(-n // (NCORES * P * G)) * G  # ceil to a multiple of G
    nc = build_nc(jr)
    in_maps = _prep_inputs(outputs, labels, NCORES, jr)
    res = bass_utils.run_bass_kernel_spmd(
        nc, in_maps, core_ids=list(range(NCORES)), trace=trace
    )
    ece = _decode([r["out"] for r in res.results], n)
    return ece, res


def kernel(outputs, labels):
    ece, _ = kernel_impl(outputs, labels)
    return ece



# revision 2
# speedup vs baseline: 7.2815x; 7.2815x over previous
"""ECE (expected calibration error) kernel for Trainium2, 8 NeuronCores.

Math (matches torch ECELoss(n_bins=20) / the jax reference):
    conf_i = max_c outputs[i, c]
    acc_i  = 1[outputs[i, labels_i] == conf_i]   (== argmax correct; exact on
             this data - verified zero tie mismatches)
    bin membership via step functions S[i, b] = conf_i > b/20, b = 0..20
    cum[b] = sum_i S[i,b] * v_i  for v in {conf, acc}
    sum_v[b] = cum[b] - cum[b+1]         (equal-width (lo, hi] bins + clip)
    ece = sum_b |sum_conf[b] - sum_acc[b]| / N

Device mapping (per core, data-parallel over samples):
    The host SORTS samples by label and packs them into cells of 16
    same-label samples, one cell per (16-partition Q7 band, row). ECE is
    permutation-invariant over samples, so this is purely a sharding/layout
    choice - but it makes the per-sample gather x[i, label_i] expressible as
    GPSIMD indirect_copy, whose per-band-uniform uint16 indices (precomputed
    host-side as t*128 + label) gather all 16 partitions of a band at once.

    Per group of G=20 tiles ([P=128, G, C=128] = 1.31 MB DMA):
      - VectorE: one batched reduce_max -> conf[P, G]; one TT is_equal
        (picked vs conf) -> acc; one broadcast TT is_gt (conf vs edges) ->
        step matrix S.  (~3.3 us)
      - GPSIMD: one indirect_copy -> picked[P, G].
      - TensorE: 2 jumbo matmuls [K=128] x ([2J] x [J*NE]) accumulating
        cum partial sums into PSUM across the whole shard.
      - DMA: 1.31 MB at ~330 GB/s/core = ~3.9 us  <- intended bottleneck.
    Host: sum the 8 cores' [2J, J*NE] partials, undo the jumbo layout,
    finish the 21->20 differencing and |.|/N.
    Padding rows are all-zero => conf = 0 => S == 0 => contribute nothing.

Built on bacc.Bacc (not raw Bass): its compile pipeline legalizes
multi-sync-wait instructions via event semaphores, which this walrus build
requires (each ISA struct carries only one sync wait).
"""

import numpy as np

P = 128          # SBUF partitions (samples per tile)
C = 128          # classes
NB = 20          # ECE bins
NE = NB + 1      # bin edges
NCORES = 8
NBANDS = 8       # Q7 cores (16 partitions each)
G = 20           # tiles per group (per DMA / per batched vector op)
J = 10           # tiles per jumbo matmul (M = 2*J <= 128, N = J*NE <= 512)


def build_nc(jr, repeat=1):
    """Build the Bass module for one core with JR rows per partition.

    repeat > 1 wraps the whole group loop in an on-device For_i that
    recomputes the same result `repeat` times (PSUM restarts each trip) -
    used only for wall-clock perf measurement via run-time deltas.
    """
    import concourse.bacc as bacc
    import concourse.mybir as mybir
    from concourse.tile import TileContext

    f32 = mybir.dt.float32
    u16 = mybir.dt.uint16
    Alu = mybir.AluOpType
    ng = jr // G
    assert jr % G == 0 and G % J == 0
    nj = G // J

    nc = bacc.Bacc("TRN2", target_bir_lowering=False)
    x = nc.dram_tensor("x", (P, jr, C), f32, kind="ExternalInput")
    edges = nc.dram_tensor("edges", (P, NE), f32, kind="ExternalInput")
    idxs = nc.dram_tensor("idxs", (P, 2 * ng), u16, kind="ExternalInput")
    out = nc.dram_tensor("out", (2 * J, NE * J), f32, kind="ExternalOutput")

    with TileContext(nc) as tc:
        with (
            tc.tile_pool(name="consts", bufs=1) as cpool,
            tc.tile_pool(name="xin", bufs=4) as xpool,
            tc.tile_pool(name="vt", bufs=3) as vpool,
            tc.tile_pool(name="pk", bufs=3) as kpool,
            tc.tile_pool(name="st", bufs=3) as spool,
            tc.tile_pool(name="res", bufs=1) as rpool,
            tc.tile_pool(name="acc", bufs=1, space="PSUM") as ppool,
        ):
            edgesb = cpool.tile([P, NE], f32)
            nc.sync.dma_start(edgesb[:], edges[:])
            idxb = cpool.tile([P, 2 * ng], u16)
            nc.sync.dma_start(idxb[:], idxs[:])

            psum = ppool.tile([2 * J, NE * J], f32)

            def group_body(g):
                xt = xpool.tile([P, G, C], f32)
                nc.sync.dma_start(xt[:], x[:, g * G:(g + 1) * G, :])

                # vt free layout: per jumbo j a contiguous [conf(J) | acc(J)]
                # block, so each matmul's stationary AP is one free dim.
                vt = vpool.tile([P, nj, 2 * J], f32)
                vt4 = vt[:].rearrange("p j (h t) -> p j h t", h=2)
                nc.vector.tensor_reduce(
                    vt4[:, :, 0, :], xt[:], axis=mybir.AxisListType.X, op=Alu.max
                )

                # picked[p, t] = x[p, t, label[band, t]]: GPSIMD gather with
                # per-band shared indices (host packs 16 same-label samples
                # per band cell).
                pk = kpool.tile([P, G], f32)
                nc.gpsimd.indirect_copy(
                    pk[:],
                    xt[:].rearrange("p g c -> p (g c)"),
                    idxb[:][:, 2 * g:2 * g + 2],
                    True,
                )

                # acc = (picked == conf); fp32 equality is exact here.
                pk3 = pk[:].rearrange("p (j t) -> p j t", j=nj)
                nc.vector.tensor_tensor(
                    vt4[:, :, 1, :], pk3, vt4[:, :, 0, :], Alu.is_equal
                )

                # S[i, t, b] = conf[i, t] > edge[b]
                st = spool.tile([P, G, NE], f32)
                st4 = st[:].rearrange("p (j t) e -> p j t e", j=nj)
                conf4 = vt4[:, :, 0, :][:, :, :, None].broadcast_to(
                    [P, nj, J, NE]
                )
                edges4 = edgesb[:][:, None, None, :].broadcast_to(
                    [P, nj, J, NE]
                )
                nc.vector.tensor_tensor(st4, conf4, edges4, Alu.is_gt)

                # PE: accumulate cum[(h,t), (t',b)] += sum_i V[i,h,t]*S[i,t',b]
                for j in range(nj):
                    nc.tensor.matmul(
                        psum[:],
                        vt[:][:, j, :],
                        st[:][:, j * J:(j + 1) * J, :],
                        start=(g == 0 and j == 0),
                        stop=(g == ng - 1 and j == nj - 1),
                    )

            if repeat > 1:
                with tc.For_i(0, repeat, 1):
                    for g in range(ng):
                        group_body(g)
            else:
                for g in range(ng):
                    group_body(g)

            res = rpool.tile([2 * J, NE * J], f32)
            nc.scalar.copy(res[:], psum[:])
            nc.sync.dma_start(out[:], res[:])

    nc.finalize()
    return nc


def _prep_inputs(outputs, labels, ncores, jr):
    """Label-sorted cell packing. Each cell = 16 samples sharing one label,
    mapped to one (core, band, row); per-label sample lists are padded with
    zero rows to a multiple of 16 so every cell is label-uniform."""
    n = outputs.shape[0]
    ng = jr // G
    ncells = ncores * NBANDS * jr
    labels = np.asarray(labels).astype(np.int64).ravel()

    order = np.argsort(labels, kind="stable")
    counts = np.bincount(labels, minlength=C)
    cells_per_label = -(-counts // 16)
    assert int(cells_per_label.sum()) <= ncells, (
        f"cells {int(cells_per_label.sum())} > capacity {ncells}; raise jr"
    )

    pidx = np.full(ncells * 16, -1, np.int64)
    celllab = np.zeros(ncells, np.uint16)
    pos_cell = 0
    pos_src = 0
    for lab in range(C):
        nl = int(counts[lab])
        cl = int(cells_per_label[lab])
        pidx[pos_cell * 16: pos_cell * 16 + nl] = order[pos_src:pos_src + nl]
        celllab[pos_cell:pos_cell + cl] = lab
        pos_cell += cl
        pos_src += nl

    xfull = np.zeros((ncells * 16, C), np.float32)
    valid = pidx >= 0
    xfull[valid] = outputs[pidx[valid]]
    # cell c -> (core, band, row); partition = band*16 + slot
    xs = np.ascontiguousarray(
        xfull.reshape(ncores, NBANDS, jr, 16, C).transpose(0, 1, 3, 2, 4)
    ).reshape(ncores, P, jr, C)
    celllab3 = celllab.reshape(ncores, NBANDS, jr)

    # indirect_copy index layout: output position t of group g reads the
    # uint16 at (partition band*16 + t%16, column 2*g + t//16).
    rows = np.arange(jr)
    gg = rows // G
    tt = rows % G
    idxs = np.zeros((ncores, P, 2 * ng), np.uint16)
    for core in range(ncores):
        for band in range(NBANDS):
            idxs[core, band * 16 + (tt % 16), 2 * gg + tt // 16] = (
                tt * C + celllab3[core, band, :]
            ).astype(np.uint16)

    edges = np.broadcast_to(
        (np.arange(NE, dtype=np.float32) / NB), (P, NE)
    ).copy()
    return [
        {"x": xs[c], "edges": edges, "idxs": idxs[c]} for c in range(ncores)
    ]


def _decode(core_outs, n):
    acc = np.zeros((2 * J, NE * J), np.float64)
    for r in core_outs:
        acc += r
    cum_conf = np.zeros(NE, np.float64)
    cum_acc = np.zeros(NE, np.float64)
    for k in range(J):
        cum_conf += acc[k, k * NE:(k + 1) * NE]
        cum_acc += acc[J + k, k * NE:(k + 1) * NE]
    sum_conf = cum_conf[:NB] - cum_conf[1:]
    sum_acc = cum_acc[:NB] - cum_acc[1:]
    ece = np.abs(sum_conf - sum_acc).sum() / n
    return np.array([ece], dtype=np.float32)


def kernel_impl(outputs, labels, trace=False):
    from concourse import bass_utils

    outputs = np.ascontiguousarray(np.asarray(outputs), dtype=np.float32)
    labels = np.asarray(labels)
    n = outputs.shape[0]
    assert outputs.shape[1] == C
    jr = -(-n // (NCORES * P * G)) * G  # ceil to a multiple of G
    nc = build_nc(jr)
    in_maps = _prep_inputs(outputs, labels, NCORES, jr)
    res = bass_utils.run_bass_kernel_spmd(
        nc, in_maps, core_ids=list(range(NCORES)), trace=trace
    )
    ece = _decode([r["out"] for r in res.results], n)
    return ece, res


def kernel(outputs, labels):
    ece, _ = kernel_impl(outputs, labels)
    return ece
